# revision 23
# baseline (speedup 1.0000x reference)
"""Trainium2 Bass kernel for CoExDispProcessor (topk_masking).

Per-sample computation (data-parallel over batch across 8 cores):
  1. top-2 over the D=48 disparity axis of cost [1,48,128,240] -> softmax
     blend of the two indices -> disp4 [128,240]
  2. 3x3 unfold of disp4 (zero pad) -> nearest 4x upsample -> weighted sum
     with softmax over the 9 channels of spg [9,512,960] -> disp1 [512,960]

v7 layout/schedule:
  - host-side transposes make every DMA contiguous per partition: cost as 4
    quarter tiles [128,48,60], spg as 4 chunk DMAs [128,9,4,240] (f32->fp16
    cast in the SWDGE datapath; full f32 HBM read).
  - spg triggers are self-paced behind the Pool den chain so the queued spg
    stream does not starve the cost quarters of HBM bandwidth (the topk is
    gated on cost arrival).
  - one Exp per chunk; den trees serial on Pool; reciprocal in-place on ACT
    with x0.25 scale folded (r4 = 4/den); num muls+adds on DVE (chunk 0
    adds on Pool); final num*r4 as 2x fp16 TT with bf16 output.
  - per-half coarse tiles (delta/texp/...) keep the dependency graph clean.
"""

import os
import sys
from contextlib import ExitStack

import numpy as np

if "/opt/trn_rl_repo" not in sys.path:
    sys.path.insert(0, "/opt/trn_rl_repo")

import concourse.bass as bass
import concourse.bacc as bacc
import concourse.tile as tile
from concourse import mybir
from concourse.bass_utils import run_bass_kernel_spmd

F32 = mybir.dt.float32
BF16 = mybir.dt.bfloat16
FP16 = mybir.dt.float16
U16 = mybir.dt.uint16
OP = mybir.AluOpType
ACT = mybir.ActivationFunctionType

B, D, H, W = 8, 48, 128, 240
HF, WF = 4 * H, 4 * W  # 512, 960
N_CORES = 8

NQ = 4  # cost quarters
WQ = W // NQ  # 60 coarse cols per quarter
HALF_W = 120  # disp4 halves (= 2 cost quarters)

NCH = 4  # fine chunks
FCH = WF // NCH  # fine columns per chunk (240)
WCH = FCH // 4  # coarse columns per chunk (60)

POOL_ADD_CHUNKS = ()  # chunks whose num-adds run on Pool
NEWTON = False


def _act_reciprocal(nc, out_ap, in_ap, scale=1.0):
    eng = nc.scalar
    return eng.add_instruction(
        mybir.InstActivation(
            name=nc.get_next_instruction_name(),
            func=ACT.Reciprocal,
            ins=[
                eng.lower_ap(in_ap),
                mybir.ImmediateValue(dtype=F32, value=0.0),
                mybir.ImmediateValue(dtype=F32, value=float(scale)),
                mybir.ImmediateValue(dtype=F32, value=0.0),
            ],
            outs=[eng.lower_ap(out_ap)],
        )
    )


def build_kernel(ctx: ExitStack, tc: tile.TileContext, out_d, cost_d, spg_d):
    nc = tc.nc

    out_v = out_d.rearrange("(R dr) (k w) -> R dr k w", dr=4, k=NCH)

    persist = ctx.enter_context(tc.tile_pool(name="persist", bufs=1))
    small = ctx.enter_context(tc.tile_pool(name="small", bufs=1))
    raw_pool = ctx.enter_context(tc.tile_pool(name="raw", bufs=6))
    e_pool = ctx.enter_context(tc.tile_pool(name="epool", bufs=NCH))
    acc_pool = ctx.enter_context(tc.tile_pool(name="accp", bufs=1))
    out_pool = ctx.enter_context(tc.tile_pool(name="outp", bufs=2))

    # ---- persistent tiles -------------------------------------------------
    ctq = [persist.tile([128, D, WQ], F32, tag=f"ctq{q}", name=f"ctq{q}")
           for q in range(NQ)]
    v8 = persist.tile([128, W, 8], F32)
    i8 = persist.tile([128, 8, W], U16)
    # per-quarter coarse tiles
    i1f = [small.tile([128, WQ], F32, tag=f"i1f{h}", name=f"i1f{h}") for h in range(4)]
    i2f = [small.tile([128, WQ], F32, tag=f"i2f{h}", name=f"i2f{h}") for h in range(4)]
    delta = [small.tile([128, WQ], F32, tag=f"dl{h}", name=f"dl{h}") for h in range(4)]
    texp = [small.tile([128, WQ], F32, tag=f"tx{h}", name=f"tx{h}") for h in range(4)]
    numc = [small.tile([128, WQ], F32, tag=f"nm{h}", name=f"nm{h}") for h in range(4)]
    denc = [small.tile([128, WQ], F32, tag=f"dn{h}", name=f"dn{h}") for h in range(4)]
    rden = [small.tile([128, WQ], F32, tag=f"rd{h}", name=f"rd{h}") for h in range(4)]
    disp4 = [small.tile([128, WQ], F32, tag=f"d4{h}", name=f"d4{h}") for h in range(4)]
    rv = []
    urep = []
    for s in range(3):
        t = small.tile([128, W + 2], F32, tag=f"rv{s}")
        rv.append(t)
        nc.vector.memset(t[:], 0.0)
        u = small.tile([128, 4 * (W + 2)], FP16, tag=f"urep{s}")
        urep.append(u)

    # per-chunk fine tiles
    e_tiles = [
        e_pool.tile([128, 9, 4, FCH], FP16, tag="e", name=f"e{_k}")
        for _k in range(NCH)
    ]
    p_dve = acc_pool.tile([128, 3, 4 * FCH], FP16, tag="p_dve")
    p_pool = acc_pool.tile([128, 3, 4 * FCH], FP16, tag="p_pool")
    dens = [
        acc_pool.tile([128, 4 * FCH], FP16, tag=f"den{k}", name=f"den{k}")
        for k in range(NCH)
    ]
    nums = [
        acc_pool.tile([128, 4 * FCH], FP16, tag=f"num{k}", name=f"num{k}")
        for k in range(NCH)
    ]

    # ---- cost quarter DMAs (contiguous per partition) -----------------------
    # On the same SWDGE ring as spg: FIFO drain order gives the cost stream
    # (which gates the whole DVE topk) full HBM bandwidth first.
    def cost_trigger(q):
        nc.gpsimd.dma_start(ctq[q][:], cost_d[:, q])

    raw_tiles = {}

    def spg_trigger_g(k, g):
        raw = raw_pool.tile(
            [128, 3, 4, FCH], FP16, tag="raw", name=f"raw{k}_{g}"
        )
        nc.gpsimd.dma_start(raw[:], spg_d[:, k, 3 * g:3 * g + 3])
        raw_tiles[(k, g)] = raw

    def spg_trigger(k):
        for g in range(3):
            spg_trigger_g(k, g)

    def exp_group(k, g):
        nc.scalar.activation(
            e_tiles[k][:, 3 * g:3 * g + 3].rearrange("p c a b -> p (c a b)"),
            raw_tiles[(k, g)][:].rearrange("p c a b -> p (c a b)"),
            ACT.Exp,
        )

    def exp_chunk(k):
        for g in range(3):
            exp_group(k, g)

    # ---- DVE: top-2 -------------------------------------------------------
    def maxes(q):
        for j in range(WQ):
            w = q * WQ + j
            nc.vector.max(out=v8[:, w], in_=ctq[q][:, :, j])
        for j in range(WQ):
            w = q * WQ + j
            nc.vector.max_index(i8[:, :, w], v8[:, w], ctq[q][:, :, j])

    def disp4_a(q):
        a, b = q * WQ, (q + 1) * WQ
        sl = slice(a, b)
        nc.vector.tensor_copy(i1f[q][:], i8[:, 0, sl])
        nc.vector.tensor_copy(i2f[q][:], i8[:, 1, sl])
        nc.vector.tensor_sub(delta[q][:], v8[:, sl, 1], v8[:, sl, 0])

    def texp_q(q):
        nc.scalar.activation(texp[q][:], delta[q][:], ACT.Exp)

    def disp4_b(q):
        a, b = q * WQ, (q + 1) * WQ
        nc.vector.tensor_scalar_add(denc[q][:], texp[q][:], 1.0)
        nc.vector.tensor_mul(numc[q][:], texp[q][:], i2f[q][:])
        nc.vector.reciprocal(rden[q][:], denc[q][:])
        nc.vector.tensor_add(numc[q][:], numc[q][:], i1f[q][:])
        nc.vector.tensor_mul(disp4[q][:], numc[q][:], rden[q][:])
        # rv[s][r, 1+w] = disp4[r + s - 1, w] for this quarter's columns
        nc.vector.tensor_copy(rv[1][:, 1 + a:1 + b], disp4[q][:])
        nc.sync.dma_start(rv[0][1:128, 1 + a:1 + b], disp4[q][0:127, :])
        nc.sync.dma_start(rv[2][0:127, 1 + a:1 + b], disp4[q][1:128, :])

    def urep_q(q):
        # piece q of urep depends only on quarter q's disp4 (+ zero pad):
        # rv cols [0,61) / [61,121) / [121,181) / [181,242)
        ra = 0 if q == 0 else 61 + 60 * (q - 1)
        rb = 61 + 60 * q if q < 3 else W + 2
        ua, ub = 4 * ra, 4 * rb
        for s in range(3):
            nc.scalar.copy(
                urep[s][:, ua:ub].rearrange("p (x dw) -> p x dw", dw=4),
                rv[s][:, ra:rb].unsqueeze(2).broadcast_to([128, rb - ra, 4]),
            )

    # ---- fine phase per chunk ---------------------------------------------
    def u4(k, c):
        cirow, cj = c // 3, c % 3
        off = 4 * (cj + k * WCH)
        return (urep[cirow][:, off:off + FCH]
                .unsqueeze(1).broadcast_to([128, 4, FCH]))

    gate = small.tile([128, 8], F32, tag="gate")

    def pool_gate():
        # orders the Pool stream behind the end of the topk (reads the last
        # MAX8 output) to test SBUF contention between Pool and DVE
        nc.gpsimd.tensor_copy(gate[:], v8[:, W - 1])

    def den_chunk(k):
        e = e_tiles[k]
        den = dens[k]
        ef = lambda c: e[:, c].rearrange("p a b -> p (a b)")
        nc.gpsimd.tensor_add(den[:], ef(0), ef(1))
        for c in range(2, 9):
            nc.gpsimd.tensor_add(den[:], den[:], ef(c))

    def recip_chunk(k):
        # in-place reciprocal with x4 folded: dens[k] <- 4/dens[k]
        _act_reciprocal(nc, dens[k][:], dens[k][:], scale=0.25)

    def num_chunk(k, eng_name="vector"):
        eng = getattr(nc, eng_name)
        e = e_tiles[k]
        num = nums[k]
        p = p_pool if eng_name == "gpsimd" else p_dve
        p3 = lambda i: p[:, i].rearrange("p (a b) -> p a b", a=4)
        eng.tensor_mul(p3(0), e[:, 0], u4(k, 0))
        eng.tensor_mul(p3(1), e[:, 1], u4(k, 1))
        eng.tensor_add(num[:], p[:, 0], p[:, 1])
        for c in range(2, 9):
            eng.tensor_mul(p3(c % 3), e[:, c], u4(k, c))
            eng.tensor_add(num[:], num[:], p[:, c % 3])

    def norm_chunk(k):
        outt = out_pool.tile([128, 4, FCH], BF16, tag="outt", name=f"outt{k}")
        nc.vector.tensor_mul(
            outt[:].rearrange("p a b -> p (a b)"), nums[k][:], dens[k][:]
        )
        nc.sync.dma_start(out_v[:, :, k, :], outt[:])

    # ---- schedule ---------------------------------------------------------
    # SWDGE ring FIFO: cost quarters first (the topk gates on them), two
    # early spg groups woven in, then the rest of the spg stream.
    cost_trigger(0)
    cost_trigger(1)
    spg_trigger_g(0, 0)
    cost_trigger(2)
    spg_trigger_g(0, 1)
    cost_trigger(3)
    spg_trigger_g(0, 2)
    spg_trigger(1)
    spg_trigger(2)
    spg_trigger(3)
    exp_chunk(0)
    den_chunk(0)
    exp_chunk(1)
    den_chunk(1)

    maxes(0)
    disp4_a(0)
    texp_q(0)
    maxes(1)
    disp4_b(0)
    urep_q(0)
    disp4_a(1)
    texp_q(1)
    exp_chunk(2)
    den_chunk(2)
    maxes(2)
    disp4_b(1)
    urep_q(1)
    disp4_a(2)
    texp_q(2)
    maxes(3)
    disp4_b(2)
    urep_q(2)
    disp4_a(3)
    texp_q(3)
    disp4_b(3)
    urep_q(3)
    exp_chunk(3)
    den_chunk(3)
    recip_chunk(0)
    recip_chunk(1)
    num_chunk(0)
    norm_chunk(0)
    num_chunk(1)
    recip_chunk(2)
    norm_chunk(1)
    num_chunk(2)
    recip_chunk(3)
    norm_chunk(2)
    num_chunk(3)
    norm_chunk(3)


def build_program():
    nc = bacc.Bacc(
        "TRN2",
        target_bir_lowering=False,
        debug=False,
        enable_asserts=False,
        num_devices=N_CORES,
    )
    # host-transposed layouts (pure layout changes, no computation):
    #   cost: [128(h), 4(q), 48(d), 60(wq)]  -- contiguous per (h, q)
    #   spg:  [128(R), 4(k), 9(c), 4(dr), 240(w)] -- contiguous per (R, k)
    cost_d = nc.dram_tensor("cost", [H, NQ, D, WQ], F32, kind="ExternalInput").ap()
    spg_d = nc.dram_tensor(
        "spg", [H, NCH, 9, 4, FCH], F32, kind="ExternalInput"
    ).ap()
    out_d = nc.dram_tensor("out", [HF, WF], BF16, kind="ExternalOutput").ap()
    with tile.TileContext(nc) as tc:
        with ExitStack() as ctx:
            build_kernel(ctx, tc, out_d, cost_d, spg_d)
    nc.compile()
    return nc


def _install_ntff_hook():
    """Provide antenv.axon_hooks + register the ctypes NTFF profiler."""
    import types

    if "antenv.axon_hooks" in sys.modules:
        return True
    try:
        import antenv
        from trn_agent_boot.trn_boot import _ntff_profile_via_ctypes

        mod = types.ModuleType("antenv.axon_hooks")
        mod._hook = None

        def set_axon_ntff_profile_hook(hook):
            mod._hook = hook

        def get_axon_ntff_profile_hook():
            return mod._hook

        mod.set_axon_ntff_profile_hook = set_axon_ntff_profile_hook
        mod.get_axon_ntff_profile_hook = get_axon_ntff_profile_hook
        sys.modules["antenv.axon_hooks"] = mod
        antenv.axon_hooks = mod
        mod._hook = _ntff_profile_via_ctypes("/opt/axon/libaxon_pjrt.so")
        return True
    except Exception as e:  # profiling is best-effort
        print(f"NTFF hook install failed: {e}")
        return False


LAST_RESULTS = None


def kernel(cost: np.ndarray, spg: np.ndarray) -> np.ndarray:
    """cost [8,1,48,128,240] f32, spg [8,9,512,960] f32 -> disp1 [8,512,960] f32."""
    global LAST_RESULTS
    cost = np.asarray(cost, dtype=np.float32)
    spg = np.asarray(spg, dtype=np.float32)
    assert cost.shape == (B, 1, D, H, W) and spg.shape == (B, 9, HF, WF)

    # host-side layout transforms (transpose only)
    cost_t = np.ascontiguousarray(
        cost.reshape(B, D, H, NQ, WQ).transpose(0, 2, 3, 1, 4)
    )  # [B, 128, 4, 48, 60]
    spg_t = np.ascontiguousarray(
        spg.reshape(B, 9, H, 4, NCH, FCH).transpose(0, 2, 4, 1, 3, 5)
    )  # [B, 128, 4, 9, 4, 240]

    nc = build_program()
    in_maps = [
        {"cost": cost_t[b], "spg": spg_t[b]} for b in range(B)
    ]
    trace = bool(int(os.environ.get("KERNEL_TRACE", "0")))
    if trace:
        trace = _install_ntff_hook()
    res = run_bass_kernel_spmd(
        nc, in_maps, core_ids=list(range(N_CORES)), trace=trace
    )
    LAST_RESULTS = res
    out = np.stack(
        [np.asarray(res.results[b]["out"]) for b in range(B)], axis=0
    )
    return out.astype(np.float32, copy=False)
